# revision 10
# baseline (speedup 1.0000x reference)
"""Trainium2 Bass kernel for the ASR GRU encoder/decoder model.

Data-parallel over batch: 8 NeuronCores x 8 samples each. All on-device
tensors use a transposed layout ([feature/gate chunk <=128 partitions,
batch free]) so the sequential GRU recurrence is TensorE
LDWEIGHTS-bound and VectorE/ScalarE stay lane-efficient.

Self-contained: only needs /opt/trn_rl_repo (the platform repo).
"""
import sys
from contextlib import ExitStack

import numpy as np

sys.path.insert(0, "/opt/trn_rl_repo")

import ml_dtypes  # noqa: E402
import concourse.bass as bass  # noqa: E402
import concourse.mybir as mybir  # noqa: E402
import concourse.tile as tile  # noqa: E402
from concourse import bacc  # noqa: E402
from concourse.bass_utils import run_bass_kernel_spmd  # noqa: E402

BF16 = mybir.dt.bfloat16
F32 = mybir.dt.float32
F32R = mybir.dt.float32r
NPBF16 = ml_dtypes.bfloat16

B, T, F, H, D, L = 64, 1024, 304, 304, 128, 128
NCORES = 8
BL = B // NCORES  # 8 samples per core
HA = H + 1  # augmented with ones-row (bias folding)
G3 = 3 * H  # 912 gate width
KC = [(0, 128), (128, 128), (256, HA - 256)]  # contraction chunks (305)
MC = [(0, 128), (128, 128), (256, H - 256)]  # per-gate M chunks (304)
CH = 128  # encoder gi production chunk, in timesteps

SIG = mybir.ActivationFunctionType.Sigmoid
TANH = mybir.ActivationFunctionType.Tanh

# dtype config: recurrence weights/h, production operands, gi storage
CFG = dict(rec_dt=F32, prod_dt=F32R, gi_dt=F32)


def build_kernel(t_steps=T, l_steps=L, ch=CH, cfg=CFG):
    rec_dt = cfg["rec_dt"]
    prod_dt = cfg["prod_dt"]
    gi_dt = cfg["gi_dt"]
    n_chunks = t_steps // ch
    chbl = ch * BL
    nc = bacc.Bacc("TRN2", target_bir_lowering=False, debug=False,
                   num_devices=NCORES)

    xt_d = nc.dram_tensor("xt", [HA, t_steps * BL], prod_dt,
                          kind="ExternalInput")
    wih_d = nc.dram_tensor("wih", [HA, G3], prod_dt, kind="ExternalInput")
    whh_d = nc.dram_tensor("whh", [HA, G3], F32, kind="ExternalInput")
    whhd_d = nc.dram_tensor("whhd", [HA, G3], F32, kind="ExternalInput")
    e2g_d = nc.dram_tensor("e2g", [D, G3], F32, kind="ExternalInput")
    linw_d = nc.dram_tensor("linw", [HA, D], F32, kind="ExternalInput")
    gi0_d = nc.dram_tensor("gi0", [128, 9 * BL], F32, kind="ExternalInput")
    id8_d = nc.dram_tensor("id8", [BL, BL], F32, kind="ExternalInput")
    ones_d = nc.dram_tensor("ones1", [1, BL], F32, kind="ExternalInput")
    out_d = nc.dram_tensor("logits", [BL, l_steps * D], F32,
                           kind="ExternalOutput")

    with ExitStack() as ctx:
        tc = ctx.enter_context(tile.TileContext(nc))
        cpool = ctx.enter_context(tc.tile_pool(name="const", bufs=1))
        gpool = ctx.enter_context(tc.tile_pool(name="gi", bufs=1))
        tpool = ctx.enter_context(tc.tile_pool(name="tmp", bufs=3))
        ps1 = ctx.enter_context(tc.tile_pool(name="ps1", bufs=1, space="PSUM"))
        ps2 = ctx.enter_context(tc.tile_pool(name="ps2", bufs=2, space="PSUM"))

        def ctile(shape, dt, name):
            return cpool.tile(shape, dt, tag=name, name=name)

        # ---- persistent SBUF tensors ----
        xtc = [[ctile([128, chbl], prod_dt, f"xtc{p}_{j}") for j in range(3)]
               for p in range(2)]
        wih_t = [ctile([128, G3], prod_dt, f"wih{j}") for j in range(3)]
        whh_t = [ctile([128, G3], rec_dt, f"whh{j}") for j in range(3)]
        whhd_t = [ctile([128, G3], rec_dt, f"whhd{j}") for j in range(3)]
        e2g_t = ctile([128, G3], rec_dt, "e2g")
        linw_t = [ctile([128, D], rec_dt, f"linw{j}") for j in range(3)]
        gi0_t = ctile([128, 9 * BL], F32, "gi0")
        id8_t = ctile([BL, BL], rec_dt, "id8")
        lout = ctile([BL, l_steps * D], F32, "lout")
        hh = [ctile([128, 3 * BL], rec_dt, f"h{p}") for p in range(2)]
        gis = [gpool.tile([128, 9 * chbl], gi_dt, tag=f"gi{p}", name=f"gi{p}")
               for p in range(2)]
        ohT_s = ctile([128, BL], rec_dt, "ohT")

        # ---- weight DMAs ----
        for j, (ks, pc) in enumerate(KC):
            nc.sync.dma_start(wih_t[j][:pc, :], wih_d[ks:ks + pc, :])
            nc.sync.dma_start(whh_t[j][:pc, :], whh_d[ks:ks + pc, :])
            nc.sync.dma_start(whhd_t[j][:pc, :], whhd_d[ks:ks + pc, :])
            nc.sync.dma_start(linw_t[j][:pc, :], linw_d[ks:ks + pc, :])
        nc.sync.dma_start(e2g_t[:, :], e2g_d[:, :])
        nc.sync.dma_start(gi0_t[:, :], gi0_d[:, :])
        nc.sync.dma_start(id8_t[:, :], id8_d[:, :])

        def dma_x_chunk(c):
            for j, (ks, pc) in enumerate(KC):
                nc.sync.dma_start(
                    xtc[c % 2][j][:pc, :],
                    xt_d[ks:ks + pc, c * chbl:(c + 1) * chbl])

        # ---- initial state: h = 0, ones in the augmented row ----
        for p in range(2):
            nc.vector.memset(hh[p][:, :], 0.0)
            nc.sync.dma_start(hh[p][48:49, 16:24], ones_d[:, :])

        # ---- gi production: one (nt, band) group = 3 MMs + 1 copy ----
        pw = min(512, chbl)
        n_nt = chbl // pw

        def prod_group(src_chunk, nt, bi):
            g, mj = bi // 3, bi % 3
            ms, mcn = MC[mj]
            dst = gis[(src_chunk) % 2]
            ps = ps2.tile([128, pw], F32, tag="prod", name="prod")
            for j, (ks, pc) in enumerate(KC):
                nc.tensor.matmul(
                    ps[:mcn, :],
                    wih_t[j][:pc, g * H + ms:g * H + ms + mcn],
                    xtc[src_chunk % 2][j][:pc, nt * pw:nt * pw + pw],
                    start=(j == 0), stop=(j == 2))
            nc.vector.tensor_copy(
                dst[:mcn, bi * chbl + nt * pw:bi * chbl + nt * pw + pw],
                ps[:mcn, :])

        prod_groups = [(nt, bi) for nt in range(n_nt) for bi in range(9)]

        # prologue: x chunk 0 + produce chunk 0
        dma_x_chunk(0)
        if n_chunks > 1:
            dma_x_chunk(1)
        for nt, bi in prod_groups:
            prod_group(0, nt, bi)

        c3 = lambda ap: ap.rearrange("p (a c) -> p a c", a=3)

        # ---- one GRU step (encoder flavor: gi from SBUF buffer) ----
        def enc_step(c, sl):
            s = c * ch + sl
            par = s % 2
            h_in, h_out = hh[par], hh[1 - par]
            gi = gis[c % 2]
            ps_r = ps1.tile([128, 3 * BL], F32, tag="ps_r", name="ps_r")
            ps_z = ps1.tile([128, 3 * BL], F32, tag="ps_z", name="ps_z")
            ps_n = ps1.tile([128, 3 * BL], F32, tag="ps_n", name="ps_n")
            for g, pst in ((0, ps_r), (1, ps_z), (2, ps_n)):
                for mj, (ms, mcn) in enumerate(MC):
                    for j, (ks, pc) in enumerate(KC):
                        nc.tensor.matmul(
                            pst[:mcn, mj * BL:(mj + 1) * BL],
                            whh_t[j][:pc, g * H + ms:g * H + ms + mcn],
                            h_in[:pc, j * BL:(j + 1) * BL],
                            start=(j == 0), stop=(j == 2))
            gi9 = gi[:, :].rearrange("p (bi c) -> p bi c", bi=9)
            t_r = tpool.tile([128, 3 * BL], F32, tag="t_r", name="t_r")
            t_z = tpool.tile([128, 3 * BL], F32, tag="t_z", name="t_z")
            t_n = tpool.tile([128, 3 * BL], F32, tag="t_n", name="t_n")
            r_s = tpool.tile([128, 3 * BL], F32, tag="r_s", name="r_s")
            z_s = tpool.tile([128, 3 * BL], F32, tag="z_s", name="z_s")
            n_s = tpool.tile([128, 3 * BL], F32, tag="n_s", name="n_s")
            d_t = tpool.tile([128, 3 * BL], F32, tag="d_t", name="d_t")
            e_t = tpool.tile([128, 3 * BL], F32, tag="e_t", name="e_t")
            nc.vector.tensor_add(c3(t_r[:, :]), c3(ps_r[:, :]),
                                 gi9[:, 0:3, sl * BL:(sl + 1) * BL])
            nc.scalar.activation(r_s[:, :], t_r[:, :], SIG)
            nc.vector.tensor_add(c3(t_z[:, :]), c3(ps_z[:, :]),
                                 gi9[:, 3:6, sl * BL:(sl + 1) * BL])
            nc.scalar.activation(z_s[:, :], t_z[:, :], SIG)
            nc.vector.tensor_mul(t_n[:, :], r_s[:, :], ps_n[:, :])
            nc.vector.tensor_add(c3(t_n[:, :]), c3(t_n[:, :]),
                                 gi9[:, 6:9, sl * BL:(sl + 1) * BL])
            nc.scalar.activation(n_s[:, :], t_n[:, :], TANH)
            nc.vector.tensor_sub(d_t[:, :], h_in[:, :], n_s[:, :])
            nc.vector.tensor_mul(e_t[:, :], z_s[:, :], d_t[:, :])
            nc.vector.tensor_add(h_out[:, 0:2 * BL], n_s[:, 0:2 * BL],
                                 e_t[:, 0:2 * BL])
            nc.vector.tensor_add(h_out[:MC[2][1], 2 * BL:3 * BL],
                                 n_s[:MC[2][1], 2 * BL:3 * BL],
                                 e_t[:MC[2][1], 2 * BL:3 * BL])

        # ---- encoder ----
        for c in range(n_chunks):
            if c + 2 < n_chunks:
                dma_x_chunk(c + 2)
            for sl in range(ch):
                enc_step(c, sl)
                if c + 1 < n_chunks:
                    for nt, bi in prod_groups[sl::ch]:
                        prod_group(c + 1, nt, bi)

        # ---- decoder ----
        for l in range(l_steps):
            par = (t_steps + l) % 2
            h_in, h_out = hh[par], hh[1 - par]
            ps_r = ps1.tile([128, 3 * BL], F32, tag="ps_r", name="ps_r")
            ps_z = ps1.tile([128, 3 * BL], F32, tag="ps_z", name="ps_z")
            ps_n = ps1.tile([128, 3 * BL], F32, tag="ps_n", name="ps_n")
            ps_ne = ps1.tile([128, 3 * BL], F32, tag="ps_ne", name="ps_ne")
            for g, pst in ((0, ps_r), (1, ps_z), (2, ps_n)):
                for mj, (ms, mcn) in enumerate(MC):
                    for j, (ks, pc) in enumerate(KC):
                        nc.tensor.matmul(
                            pst[:mcn, mj * BL:(mj + 1) * BL],
                            whhd_t[j][:pc, g * H + ms:g * H + ms + mcn],
                            h_in[:pc, j * BL:(j + 1) * BL],
                            start=(j == 0),
                            stop=(j == 2 and (l == 0 or g == 2)))
                    if l > 0 and g < 2:
                        nc.tensor.matmul(
                            pst[:mcn, mj * BL:(mj + 1) * BL],
                            e2g_t[:, g * H + ms:g * H + ms + mcn],
                            ohT_s[:, :],
                            start=False, stop=True)
            if l > 0:
                for mj, (ms, mcn) in enumerate(MC):
                    nc.tensor.matmul(
                        ps_ne[:mcn, mj * BL:(mj + 1) * BL],
                        e2g_t[:, 2 * H + ms:2 * H + ms + mcn],
                        ohT_s[:, :],
                        start=True, stop=True)

            t_r = tpool.tile([128, 3 * BL], F32, tag="t_r", name="t_r")
            t_z = tpool.tile([128, 3 * BL], F32, tag="t_z", name="t_z")
            t_n = tpool.tile([128, 3 * BL], F32, tag="t_n", name="t_n")
            r_s = tpool.tile([128, 3 * BL], F32, tag="r_s", name="r_s")
            z_s = tpool.tile([128, 3 * BL], F32, tag="z_s", name="z_s")
            n_s = tpool.tile([128, 3 * BL], F32, tag="n_s", name="n_s")
            d_t = tpool.tile([128, 3 * BL], F32, tag="d_t", name="d_t")
            e_t = tpool.tile([128, 3 * BL], F32, tag="e_t", name="e_t")
            if l == 0:
                nc.vector.tensor_add(t_r[:, :], ps_r[:, :], gi0_t[:, 0:24])
                nc.scalar.activation(r_s[:, :], t_r[:, :], SIG)
                nc.vector.tensor_add(t_z[:, :], ps_z[:, :], gi0_t[:, 24:48])
                nc.scalar.activation(z_s[:, :], t_z[:, :], SIG)
                nc.vector.tensor_mul(t_n[:, :], r_s[:, :], ps_n[:, :])
                nc.vector.tensor_add(t_n[:, :], t_n[:, :], gi0_t[:, 48:72])
            else:
                nc.scalar.activation(r_s[:, :], ps_r[:, :], SIG)
                nc.scalar.activation(z_s[:, :], ps_z[:, :], SIG)
                nc.vector.tensor_mul(t_n[:, :], r_s[:, :], ps_n[:, :])
                nc.vector.tensor_add(t_n[:, :], t_n[:, :], ps_ne[:, :])
            nc.scalar.activation(n_s[:, :], t_n[:, :], TANH)
            nc.vector.tensor_sub(d_t[:, :], h_in[:, :], n_s[:, :])
            nc.vector.tensor_mul(e_t[:, :], z_s[:, :], d_t[:, :])
            nc.vector.tensor_add(h_out[:, 0:2 * BL], n_s[:, 0:2 * BL],
                                 e_t[:, 0:2 * BL])
            nc.vector.tensor_add(h_out[:MC[2][1], 2 * BL:3 * BL],
                                 n_s[:MC[2][1], 2 * BL:3 * BL],
                                 e_t[:MC[2][1], 2 * BL:3 * BL])

            # logits = h_out_aug @ linW_aug.T  -> [BL, D]
            ps_log = ps1.tile([BL, D], F32, tag="ps_log", name="ps_log")
            for j, (ks, pc) in enumerate(KC):
                nc.tensor.matmul(ps_log[:, :], h_out[:pc, j * BL:(j + 1) * BL],
                                 linw_t[j][:pc, :], start=(j == 0),
                                 stop=(j == 2))
            nc.vector.tensor_copy(lout[:, l * D:(l + 1) * D], ps_log[:, :])
            if l + 1 < l_steps:
                max8 = tpool.tile([BL, 8], F32, tag="max8", name="max8")
                oh_s = tpool.tile([BL, D], rec_dt, tag="oh", name="oh")
                nc.vector.max(max8[:, :], lout[:, l * D:(l + 1) * D])
                nc.vector.tensor_scalar(
                    oh_s[:, :], lout[:, l * D:(l + 1) * D], max8[:, 0:1],
                    None, op0=mybir.AluOpType.is_equal)
                ps_oht = ps1.tile([128, BL], F32, tag="ps_oht", name="ps_oht")
                nc.tensor.transpose(ps_oht[:, :], oh_s[:, :], id8_t[:, :])
                nc.vector.tensor_copy(ohT_s[:, :], ps_oht[:, :])

        nc.sync.dma_start(out_d[:, :], lout[:, :])

    nc.compile()
    return nc


def prep_inputs(x, target, emb, enc_Wih, enc_Whh, enc_bih, enc_bhh,
                dec_Wih, dec_Whh, dec_bih, dec_bhh, lin_W, lin_b,
                t_steps=T, l_steps=L):
    """Build per-core input maps (host-side sharding + layout)."""
    x = np.asarray(x, np.float32)
    target = np.asarray(target)
    emb = np.asarray(emb, np.float32)
    enc_Wih = np.asarray(enc_Wih, np.float32)
    enc_Whh = np.asarray(enc_Whh, np.float32)
    enc_bih = np.asarray(enc_bih, np.float32)
    enc_bhh = np.asarray(enc_bhh, np.float32)
    dec_Wih = np.asarray(dec_Wih, np.float32)
    dec_Whh = np.asarray(dec_Whh, np.float32)
    dec_bih = np.asarray(dec_bih, np.float32)
    dec_bhh = np.asarray(dec_bhh, np.float32)
    lin_W = np.asarray(lin_W, np.float32)
    lin_b = np.asarray(lin_b, np.float32)

    bias_rzn = enc_bih.copy()
    bias_rzn[:2 * H] += enc_bhh[:2 * H]
    wihT = np.concatenate([enc_Wih.T, bias_rzn[None, :]], 0)
    whh_aug = np.zeros((1, G3), np.float32)
    whh_aug[0, 2 * H:] = enc_bhh[2 * H:]
    whhT = np.concatenate([enc_Whh.T, whh_aug], 0)
    whhd_aug = np.zeros((1, G3), np.float32)
    whhd_aug[0, 2 * H:] = dec_bhh[2 * H:]
    whhdT = np.concatenate([dec_Whh.T, whhd_aug], 0)
    dbias = dec_bih.copy()
    dbias[:2 * H] += dec_bhh[:2 * H]
    e2g_f32 = (emb.astype(np.float64) @ dec_Wih.T.astype(np.float64)
               + dbias.astype(np.float64)).astype(np.float32)
    linT = np.concatenate([lin_W.T, lin_b[None, :]], 0)
    id8 = np.eye(BL, dtype=np.float32)

    in_maps = []
    for cix in range(NCORES):
        bs = slice(cix * BL, (cix + 1) * BL)
        xa = x[bs, :t_steps, :]  # [BL, T, F]
        xa_t = np.ascontiguousarray(xa.transpose(2, 1, 0)).reshape(F, -1)
        xa_t = np.concatenate(
            [xa_t, np.ones((1, t_steps * BL), np.float32)], 0)
        toks = target[bs, 0, 0].astype(np.int64)
        gi0_full = e2g_f32[toks]  # [BL, G3]
        gi0 = np.zeros((128, 9 * BL), np.float32)
        for g in range(3):
            for mj, (ms, mcn) in enumerate(MC):
                gi0[:mcn, (g * 3 + mj) * BL:(g * 3 + mj + 1) * BL] = \
                    gi0_full[:, g * H + ms:g * H + ms + mcn].T
        in_maps.append({
            "xt": np.ascontiguousarray(xa_t),
            "wih": wihT, "whh": whhT, "whhd": whhdT,
            "e2g": e2g_f32, "linw": linT,
            "gi0": gi0, "id8": id8,
            "ones1": np.ones((1, BL), np.float32),
        })
    return in_maps


_NC_CACHE = {}


def run_model(x, target, emb, enc_Wih, enc_Whh, enc_bih, enc_bhh,
              dec_Wih, dec_Whh, dec_bih, dec_bhh, lin_W, lin_b,
              t_steps=T, l_steps=L, trace=False):
    key = (t_steps, l_steps)
    if key not in _NC_CACHE:
        _NC_CACHE[key] = build_kernel(t_steps, l_steps)
    nc = _NC_CACHE[key]
    in_maps = prep_inputs(x, target, emb, enc_Wih, enc_Whh, enc_bih, enc_bhh,
                          dec_Wih, dec_Whh, dec_bih, dec_bhh, lin_W, lin_b,
                          t_steps=t_steps, l_steps=l_steps)
    res = run_bass_kernel_spmd(nc, in_maps, core_ids=list(range(NCORES)),
                               trace=trace)
    logits = np.stack([res.results[i]["logits"] for i in range(NCORES)])
    logits = logits.reshape(NCORES * BL, l_steps, D).astype(np.float32)
    return logits, res


def kernel(x, target, emb, enc_Wih, enc_Whh, enc_bih, enc_bhh,
           dec_Wih, dec_Whh, dec_bih, dec_bhh, lin_W, lin_b):
    target = np.asarray(target)
    logits, _ = run_model(x, target, emb, enc_Wih, enc_Whh, enc_bih, enc_bhh,
                          dec_Wih, dec_Whh, dec_bih, dec_bhh, lin_W, lin_b)
    softmaxs = logits  # [B, L, D]
    softmax_cal = softmaxs[:, :-1, :].reshape(-1, D)
    target_cal = target[:, 1:, :].reshape(-1)
    asr_outputs = np.argmax(softmaxs[:, :-1, :], axis=2)[:, :, None]
    asr_outputs = asr_outputs.astype(np.int32)
    return softmax_cal, target_cal, asr_outputs


# revision 23
# speedup vs baseline: 1.9308x; 1.9308x over previous
"""Trainium2 Bass kernel for the ASR GRU encoder/decoder model.

Data-parallel over batch: 8 NeuronCores x 8 samples each. All on-device
tensors use a transposed layout ([feature/gate chunk <=128 partitions,
batch free]) so the sequential GRU recurrence is TensorE
LDWEIGHTS-bound and VectorE/ScalarE stay lane-efficient.

Self-contained: only needs /opt/trn_rl_repo (the platform repo).
"""
import sys
from contextlib import ExitStack

import numpy as np

sys.path.insert(0, "/opt/trn_rl_repo")

import ml_dtypes  # noqa: E402
import concourse.bass as bass  # noqa: E402
import concourse.mybir as mybir  # noqa: E402
import concourse.tile as tile  # noqa: E402
from concourse import bacc  # noqa: E402
from concourse.bass_utils import run_bass_kernel_spmd  # noqa: E402

BF16 = mybir.dt.bfloat16
F32 = mybir.dt.float32
F32R = mybir.dt.float32r
NPBF16 = ml_dtypes.bfloat16

B, T, F, H, D, L = 64, 1024, 304, 304, 128, 128
NCORES = 8
BL = B // NCORES  # 8 samples per core
HA = H + 1  # augmented with ones-row (bias folding)
G3 = 3 * H  # 912 gate width
KC = [(0, 128), (128, 128), (256, HA - 256)]  # contraction chunks (305)
MC = [(0, 128), (128, 128), (256, H - 256)]  # per-gate M chunks (304)
CH = 128  # encoder gi production chunk, in timesteps

SIG = mybir.ActivationFunctionType.Sigmoid
TANH = mybir.ActivationFunctionType.Tanh

# dtype config: encoder (weights/h/gi/x) vs decoder (always fp32)
CFG = dict(enc_dt=BF16)


def build_kernel(t_steps=T, l_steps=L, ch=CH, cfg=CFG):
    enc_dt = cfg["enc_dt"]
    n_chunks = t_steps // ch
    chbl = ch * BL
    nc = bacc.Bacc("TRN2", target_bir_lowering=False, debug=False,
                   num_devices=NCORES)

    xt_d = nc.dram_tensor("xt", [HA, t_steps * BL], enc_dt,
                          kind="ExternalInput")
    wih_d = nc.dram_tensor("wih", [HA, G3], enc_dt, kind="ExternalInput")
    whh_d = nc.dram_tensor("whh", [HA, G3], enc_dt, kind="ExternalInput")
    whhd_d = nc.dram_tensor("whhd", [HA, G3], F32, kind="ExternalInput")
    e2g_d = nc.dram_tensor("e2g", [D, G3], F32, kind="ExternalInput")
    linw_d = nc.dram_tensor("linw", [HA, D], F32, kind="ExternalInput")
    gi0_d = nc.dram_tensor("gi0", [128, 9 * BL], F32, kind="ExternalInput")
    id8_d = nc.dram_tensor("id8", [BL, BL], F32, kind="ExternalInput")
    ones_d = nc.dram_tensor("ones1", [1, BL], F32, kind="ExternalInput")
    onesb_d = nc.dram_tensor("ones1b", [1, BL], enc_dt, kind="ExternalInput")
    out_d = nc.dram_tensor("logits", [BL, l_steps * D], F32,
                           kind="ExternalOutput")

    with ExitStack() as ctx:
        tc = ctx.enter_context(tile.TileContext(nc))
        cpool = ctx.enter_context(tc.tile_pool(name="const", bufs=1))
        gpool = ctx.enter_context(tc.tile_pool(name="gi", bufs=1))
        tpool = ctx.enter_context(tc.tile_pool(name="tmp", bufs=3))
        ps1 = ctx.enter_context(tc.tile_pool(name="ps1", bufs=1, space="PSUM"))
        ps2 = ctx.enter_context(tc.tile_pool(name="ps2", bufs=2, space="PSUM"))

        def ctile(shape, dt, name):
            return cpool.tile(shape, dt, tag=name, name=name)

        # ---- persistent SBUF tensors ----
        xtc = [[ctile([128, chbl], enc_dt, f"xtc{p}_{j}") for j in range(3)]
               for p in range(2)]
        wih_t = [ctile([128, G3], enc_dt, f"wih{j}") for j in range(3)]
        whh_t = [ctile([128, G3], enc_dt, f"whh{j}") for j in range(3)]
        whhd_t = [ctile([128, G3], F32, f"whhd{j}") for j in range(3)]
        e2g_t = ctile([128, G3], F32, "e2g")
        linw_t = [ctile([128, D], F32, f"linw{j}") for j in range(3)]
        gi0_t = ctile([128, 9 * BL], F32, "gi0")
        id8_t = ctile([BL, BL], F32, "id8")
        lout = ctile([BL, l_steps * D], F32, "lout")
        hh = [ctile([128, 3 * BL], enc_dt, f"h{p}") for p in range(2)]
        hhd = [ctile([128, 3 * BL], F32, f"hd{p}") for p in range(2)]
        gis = [gpool.tile([128, 9 * chbl], enc_dt, tag=f"gi{p}", name=f"gi{p}")
               for p in range(2)]
        ohT_s = ctile([128, BL], F32, "ohT")

        # ---- weight DMAs ----
        for j, (ks, pc) in enumerate(KC):
            nc.sync.dma_start(wih_t[j][:pc, :], wih_d[ks:ks + pc, :])
            nc.sync.dma_start(whh_t[j][:pc, :], whh_d[ks:ks + pc, :])
            nc.sync.dma_start(whhd_t[j][:pc, :], whhd_d[ks:ks + pc, :])
            nc.sync.dma_start(linw_t[j][:pc, :], linw_d[ks:ks + pc, :])
        nc.sync.dma_start(e2g_t[:, :], e2g_d[:, :])
        nc.sync.dma_start(gi0_t[:, :], gi0_d[:, :])
        nc.sync.dma_start(id8_t[:, :], id8_d[:, :])

        def dma_x_chunk(c):
            for j, (ks, pc) in enumerate(KC):
                nc.sync.dma_start(
                    xtc[c % 2][j][:pc, :],
                    xt_d[ks:ks + pc, c * chbl:(c + 1) * chbl])

        # ---- initial state: h = 0, ones in the augmented row ----
        for p in range(2):
            nc.vector.memset(hh[p][:, :], 0.0)
            nc.sync.dma_start(hh[p][48:49, 16:24], onesb_d[:, :])
            nc.vector.memset(hhd[p][:, :], 0.0)
            nc.sync.dma_start(hhd[p][48:49, 16:24], ones_d[:, :])

        # ---- gi production: one (nt, band) group = 3 MMs + 1 copy ----
        pw = min(512, chbl)
        n_nt = chbl // pw

        def prod_group(src_chunk, nt, bi):
            g, mj = bi // 3, bi % 3
            ms, mcn = MC[mj]
            dst = gis[(src_chunk) % 2]
            ps = ps2.tile([128, pw], F32, tag="prod", name="prod")
            for j, (ks, pc) in enumerate(KC):
                nc.tensor.matmul(
                    ps[:mcn, :],
                    wih_t[j][:pc, g * H + ms:g * H + ms + mcn],
                    xtc[src_chunk % 2][j][:pc, nt * pw:nt * pw + pw],
                    start=(j == 0), stop=(j == 2))
            nc.vector.tensor_copy(
                dst[:mcn, bi * chbl + nt * pw:bi * chbl + nt * pw + pw],
                ps[:mcn, :])

        prod_groups = [(nt, bi) for nt in range(n_nt) for bi in range(9)]

        # prologue: x chunk 0 + produce chunk 0
        dma_x_chunk(0)
        if n_chunks > 1:
            dma_x_chunk(1)
        for nt, bi in prod_groups:
            prod_group(0, nt, bi)

        c3 = lambda ap: ap.rearrange("p (a c) -> p a c", a=3)

        # ---- one GRU step (encoder flavor: gi from SBUF buffer) ----
        def enc_step(c, sl):
            s = c * ch + sl
            par = s % 2
            h_in, h_out = hh[par], hh[1 - par]
            gi = gis[c % 2]
            ps_r = ps1.tile([128, 3 * BL], F32, tag="ps_r", name="ps_r")
            ps_z = ps1.tile([128, 3 * BL], F32, tag="ps_z", name="ps_z")
            ps_n = ps1.tile([128, 3 * BL], F32, tag="ps_n", name="ps_n")
            # gate order r, n, z: the n-chain (longest) starts 1/3 earlier,
            # and z's sigmoid finishes while the n-chain runs
            for g, pst in ((0, ps_r), (2, ps_n), (1, ps_z)):
                for mj, (ms, mcn) in enumerate(MC):
                    for j, (ks, pc) in enumerate(KC):
                        nc.tensor.matmul(
                            pst[:mcn, mj * BL:(mj + 1) * BL],
                            whh_t[j][:pc, g * H + ms:g * H + ms + mcn],
                            h_in[:pc, j * BL:(j + 1) * BL],
                            start=(j == 0), stop=(j == 2))
            gi9 = gi[:, :].rearrange("p (bi c) -> p bi c", bi=9)
            t_r = tpool.tile([128, 3 * BL], F32, tag="t_r", name="t_r")
            t_z = tpool.tile([128, 3 * BL], F32, tag="t_z", name="t_z")
            t_n = tpool.tile([128, 3 * BL], F32, tag="t_n", name="t_n")
            r_s = tpool.tile([128, 3 * BL], F32, tag="r_s", name="r_s")
            z_s = tpool.tile([128, 3 * BL], F32, tag="z_s", name="z_s")
            n_s = tpool.tile([128, 3 * BL], F32, tag="n_s", name="n_s")
            d_t = tpool.tile([128, 3 * BL], F32, tag="d_t", name="d_t")
            e_t = tpool.tile([128, 3 * BL], F32, tag="e_t", name="e_t")
            nc.vector.tensor_add(c3(t_r[:, :]), c3(ps_r[:, :]),
                                 gi9[:, 0:3, sl * BL:(sl + 1) * BL])
            nc.scalar.activation(r_s[:, :], t_r[:, :], SIG)
            nc.vector.tensor_mul(t_n[:, :], r_s[:, :], ps_n[:, :])
            nc.vector.tensor_add(c3(t_n[:, :]), c3(t_n[:, :]),
                                 gi9[:, 6:9, sl * BL:(sl + 1) * BL])
            nc.scalar.activation(n_s[:, :], t_n[:, :], TANH)
            nc.vector.tensor_sub(d_t[:, :], h_in[:, :], n_s[:, :])
            nc.vector.tensor_add(c3(t_z[:, :]), c3(ps_z[:, :]),
                                 gi9[:, 3:6, sl * BL:(sl + 1) * BL])
            nc.scalar.activation(z_s[:, :], t_z[:, :], SIG)
            nc.vector.tensor_mul(e_t[:, :], z_s[:, :], d_t[:, :])
            nc.vector.tensor_add(h_out[:, 0:2 * BL], n_s[:, 0:2 * BL],
                                 e_t[:, 0:2 * BL])
            nc.vector.tensor_add(h_out[:MC[2][1], 2 * BL:3 * BL],
                                 n_s[:MC[2][1], 2 * BL:3 * BL],
                                 e_t[:MC[2][1], 2 * BL:3 * BL])

        # ---- encoder ----
        for c in range(n_chunks):
            if c + 2 < n_chunks:
                dma_x_chunk(c + 2)
            for sl in range(ch):
                enc_step(c, sl)
                if c + 1 < n_chunks:
                    for nt, bi in prod_groups[sl::ch]:
                        prod_group(c + 1, nt, bi)

        # ---- decoder (fp32): hand off encoder h into f32 tiles ----
        nc.vector.tensor_copy(hhd[0][:, :], hh[t_steps % 2][:, :])
        for l in range(l_steps):
            par = l % 2
            h_in, h_out = hhd[par], hhd[1 - par]
            ps_r = ps1.tile([128, 3 * BL], F32, tag="ps_r", name="ps_r")
            ps_z = ps1.tile([128, 3 * BL], F32, tag="ps_z", name="ps_z")
            ps_n = ps1.tile([128, 3 * BL], F32, tag="ps_n", name="ps_n")
            ps_ne = ps1.tile([128, 3 * BL], F32, tag="ps_ne", name="ps_ne")
            # n-gate gh first: 9 matmuls with no one-hot dependency cover the
            # previous step's argmax->one-hot chain latency
            for g, pst in ((2, ps_n), (0, ps_r), (1, ps_z)):
                for mj, (ms, mcn) in enumerate(MC):
                    for j, (ks, pc) in enumerate(KC):
                        nc.tensor.matmul(
                            pst[:mcn, mj * BL:(mj + 1) * BL],
                            whhd_t[j][:pc, g * H + ms:g * H + ms + mcn],
                            h_in[:pc, j * BL:(j + 1) * BL],
                            start=(j == 0),
                            stop=(j == 2 and (l == 0 or g == 2)))
                    if l > 0 and g < 2:
                        nc.tensor.matmul(
                            pst[:mcn, mj * BL:(mj + 1) * BL],
                            e2g_t[:, g * H + ms:g * H + ms + mcn],
                            ohT_s[:, :],
                            start=False, stop=True)
                if l > 0 and g == 0:
                    for mj, (ms, mcn) in enumerate(MC):
                        nc.tensor.matmul(
                            ps_ne[:mcn, mj * BL:(mj + 1) * BL],
                            e2g_t[:, 2 * H + ms:2 * H + ms + mcn],
                            ohT_s[:, :],
                            start=True, stop=True)

            t_r = tpool.tile([128, 3 * BL], F32, tag="t_r", name="t_r")
            t_z = tpool.tile([128, 3 * BL], F32, tag="t_z", name="t_z")
            t_n = tpool.tile([128, 3 * BL], F32, tag="t_n", name="t_n")
            r_s = tpool.tile([128, 3 * BL], F32, tag="r_s", name="r_s")
            z_s = tpool.tile([128, 3 * BL], F32, tag="z_s", name="z_s")
            n_s = tpool.tile([128, 3 * BL], F32, tag="n_s", name="n_s")
            d_t = tpool.tile([128, 3 * BL], F32, tag="d_t", name="d_t")
            e_t = tpool.tile([128, 3 * BL], F32, tag="e_t", name="e_t")
            if l == 0:
                nc.vector.tensor_add(t_r[:, :], ps_r[:, :], gi0_t[:, 0:24])
                nc.scalar.activation(r_s[:, :], t_r[:, :], SIG)
                nc.vector.tensor_mul(t_n[:, :], r_s[:, :], ps_n[:, :])
                nc.vector.tensor_add(t_n[:, :], t_n[:, :], gi0_t[:, 48:72])
                nc.scalar.activation(n_s[:, :], t_n[:, :], TANH)
                nc.vector.tensor_sub(d_t[:, :], h_in[:, :], n_s[:, :])
                nc.vector.tensor_add(t_z[:, :], ps_z[:, :], gi0_t[:, 24:48])
                nc.scalar.activation(z_s[:, :], t_z[:, :], SIG)
            else:
                nc.scalar.activation(r_s[:, :], ps_r[:, :], SIG)
                nc.vector.tensor_mul(t_n[:, :], r_s[:, :], ps_n[:, :])
                nc.vector.tensor_add(t_n[:, :], t_n[:, :], ps_ne[:, :])
                nc.scalar.activation(n_s[:, :], t_n[:, :], TANH)
                nc.vector.tensor_sub(d_t[:, :], h_in[:, :], n_s[:, :])
                nc.scalar.activation(z_s[:, :], ps_z[:, :], SIG)
            nc.vector.tensor_mul(e_t[:, :], z_s[:, :], d_t[:, :])
            nc.vector.tensor_add(h_out[:, 0:2 * BL], n_s[:, 0:2 * BL],
                                 e_t[:, 0:2 * BL])
            nc.vector.tensor_add(h_out[:MC[2][1], 2 * BL:3 * BL],
                                 n_s[:MC[2][1], 2 * BL:3 * BL],
                                 e_t[:MC[2][1], 2 * BL:3 * BL])

            # logits = h_out_aug @ linW_aug.T  -> [BL, D]
            ps_log = ps1.tile([BL, D], F32, tag="ps_log", name="ps_log")
            for j, (ks, pc) in enumerate(KC):
                nc.tensor.matmul(ps_log[:, :], h_out[:pc, j * BL:(j + 1) * BL],
                                 linw_t[j][:pc, :], start=(j == 0),
                                 stop=(j == 2))
            nc.vector.tensor_copy(lout[:, l * D:(l + 1) * D], ps_log[:, :])
            if l + 1 < l_steps:
                max8 = tpool.tile([BL, 8], F32, tag="max8", name="max8")
                oh_s = tpool.tile([BL, D], F32, tag="oh", name="oh")
                nc.vector.max(max8[:, :], lout[:, l * D:(l + 1) * D])
                nc.vector.tensor_scalar(
                    oh_s[:, :], lout[:, l * D:(l + 1) * D], max8[:, 0:1],
                    None, op0=mybir.AluOpType.is_equal)
                ps_oht = ps1.tile([128, BL], F32, tag="ps_oht", name="ps_oht")
                nc.tensor.transpose(ps_oht[:, :], oh_s[:, :], id8_t[:, :])
                nc.vector.tensor_copy(ohT_s[:, :], ps_oht[:, :])

        nc.sync.dma_start(out_d[:, :], lout[:, :])

    nc.compile()
    return nc


def prep_inputs(x, target, emb, enc_Wih, enc_Whh, enc_bih, enc_bhh,
                dec_Wih, dec_Whh, dec_bih, dec_bhh, lin_W, lin_b,
                t_steps=T, l_steps=L):
    """Build per-core input maps (host-side sharding + layout)."""
    x = np.asarray(x, np.float32)
    target = np.asarray(target)
    emb = np.asarray(emb, np.float32)
    enc_Wih = np.asarray(enc_Wih, np.float32)
    enc_Whh = np.asarray(enc_Whh, np.float32)
    enc_bih = np.asarray(enc_bih, np.float32)
    enc_bhh = np.asarray(enc_bhh, np.float32)
    dec_Wih = np.asarray(dec_Wih, np.float32)
    dec_Whh = np.asarray(dec_Whh, np.float32)
    dec_bih = np.asarray(dec_bih, np.float32)
    dec_bhh = np.asarray(dec_bhh, np.float32)
    lin_W = np.asarray(lin_W, np.float32)
    lin_b = np.asarray(lin_b, np.float32)

    bias_rzn = enc_bih.copy()
    bias_rzn[:2 * H] += enc_bhh[:2 * H]
    wihT = np.concatenate([enc_Wih.T, bias_rzn[None, :]], 0)
    whh_aug = np.zeros((1, G3), np.float32)
    whh_aug[0, 2 * H:] = enc_bhh[2 * H:]
    whhT = np.concatenate([enc_Whh.T, whh_aug], 0)
    whhd_aug = np.zeros((1, G3), np.float32)
    whhd_aug[0, 2 * H:] = dec_bhh[2 * H:]
    whhdT = np.concatenate([dec_Whh.T, whhd_aug], 0)
    dbias = dec_bih.copy()
    dbias[:2 * H] += dec_bhh[:2 * H]
    e2g_f32 = (emb.astype(np.float64) @ dec_Wih.T.astype(np.float64)
               + dbias.astype(np.float64)).astype(np.float32)
    linT = np.concatenate([lin_W.T, lin_b[None, :]], 0)
    id8 = np.eye(BL, dtype=np.float32)

    in_maps = []
    for cix in range(NCORES):
        bs = slice(cix * BL, (cix + 1) * BL)
        xa = x[bs, :t_steps, :]  # [BL, T, F]
        xa_t = np.ascontiguousarray(xa.transpose(2, 1, 0)).reshape(F, -1)
        xa_t = np.concatenate(
            [xa_t, np.ones((1, t_steps * BL), np.float32)], 0)
        toks = target[bs, 0, 0].astype(np.int64)
        gi0_full = e2g_f32[toks]  # [BL, G3]
        gi0 = np.zeros((128, 9 * BL), np.float32)
        for g in range(3):
            for mj, (ms, mcn) in enumerate(MC):
                gi0[:mcn, (g * 3 + mj) * BL:(g * 3 + mj + 1) * BL] = \
                    gi0_full[:, g * H + ms:g * H + ms + mcn].T
        in_maps.append({
            "xt": np.ascontiguousarray(xa_t).astype(NPBF16),
            "wih": wihT.astype(NPBF16), "whh": whhT.astype(NPBF16),
            "whhd": whhdT,
            "e2g": e2g_f32, "linw": linT,
            "gi0": gi0, "id8": id8,
            "ones1": np.ones((1, BL), np.float32),
            "ones1b": np.ones((1, BL), NPBF16),
        })
    return in_maps


_NC_CACHE = {}


def run_model(x, target, emb, enc_Wih, enc_Whh, enc_bih, enc_bhh,
              dec_Wih, dec_Whh, dec_bih, dec_bhh, lin_W, lin_b,
              t_steps=T, l_steps=L, trace=False):
    key = (t_steps, l_steps)
    if key not in _NC_CACHE:
        _NC_CACHE[key] = build_kernel(t_steps, l_steps)
    nc = _NC_CACHE[key]
    in_maps = prep_inputs(x, target, emb, enc_Wih, enc_Whh, enc_bih, enc_bhh,
                          dec_Wih, dec_Whh, dec_bih, dec_bhh, lin_W, lin_b,
                          t_steps=t_steps, l_steps=l_steps)
    res = run_bass_kernel_spmd(nc, in_maps, core_ids=list(range(NCORES)),
                               trace=trace)
    logits = np.stack([res.results[i]["logits"] for i in range(NCORES)])
    logits = logits.reshape(NCORES * BL, l_steps, D).astype(np.float32)
    return logits, res


def kernel(x, target, emb, enc_Wih, enc_Whh, enc_bih, enc_bhh,
           dec_Wih, dec_Whh, dec_bih, dec_bhh, lin_W, lin_b):
    target = np.asarray(target)
    logits, _ = run_model(x, target, emb, enc_Wih, enc_Whh, enc_bih, enc_bhh,
                          dec_Wih, dec_Whh, dec_bih, dec_bhh, lin_W, lin_b)
    softmaxs = logits  # [B, L, D]
    softmax_cal = softmaxs[:, :-1, :].reshape(-1, D)
    target_cal = target[:, 1:, :].reshape(-1)
    asr_outputs = np.argmax(softmaxs[:, :-1, :], axis=2)[:, :, None]
    asr_outputs = asr_outputs.astype(np.int32)
    return softmax_cal, target_cal, asr_outputs


# revision 24
# speedup vs baseline: 2.3863x; 1.2359x over previous
"""Trainium2 Bass kernel for the ASR GRU encoder/decoder model.

Data-parallel over batch: 8 NeuronCores x 8 samples each. All on-device
tensors use a transposed layout ([feature/gate chunk <=128 partitions,
batch free]) so the sequential GRU recurrence is TensorE
LDWEIGHTS-bound and VectorE/ScalarE stay lane-efficient.

Self-contained: only needs /opt/trn_rl_repo (the platform repo).
"""
import sys
from contextlib import ExitStack

import numpy as np

sys.path.insert(0, "/opt/trn_rl_repo")

import ml_dtypes  # noqa: E402
import concourse.bass as bass  # noqa: E402
import concourse.mybir as mybir  # noqa: E402
import concourse.tile as tile  # noqa: E402
from concourse import bacc  # noqa: E402
from concourse.bass_utils import run_bass_kernel_spmd  # noqa: E402

BF16 = mybir.dt.bfloat16
F32 = mybir.dt.float32
F32R = mybir.dt.float32r
NPBF16 = ml_dtypes.bfloat16

B, T, F, H, D, L = 64, 1024, 304, 304, 128, 128
NCORES = 8
BL = B // NCORES  # 8 samples per core
HA = H + 1  # augmented with ones-row (bias folding)
G3 = 3 * H  # 912 gate width
KC = [(0, 128), (128, 128), (256, HA - 256)]  # contraction chunks (305)
MC = [(0, 128), (128, 128), (256, H - 256)]  # per-gate M chunks (304)
CH = 128  # encoder gi production chunk, in timesteps

SIG = mybir.ActivationFunctionType.Sigmoid
TANH = mybir.ActivationFunctionType.Tanh

# dtype config: encoder (weights/h/gi/x) vs decoder (always fp32)
CFG = dict(enc_dt=BF16)


def build_kernel(t_steps=T, l_steps=L, ch=CH, cfg=CFG):
    enc_dt = cfg["enc_dt"]
    n_chunks = t_steps // ch
    chbl = ch * BL
    nc = bacc.Bacc("TRN2", target_bir_lowering=False, debug=False,
                   num_devices=NCORES)

    xt_d = nc.dram_tensor("xt", [HA, t_steps * BL], enc_dt,
                          kind="ExternalInput")
    wih_d = nc.dram_tensor("wih", [HA, G3], enc_dt, kind="ExternalInput")
    whh_d = nc.dram_tensor("whh", [HA, G3], enc_dt, kind="ExternalInput")
    whhd_d = nc.dram_tensor("whhd", [HA, G3], F32, kind="ExternalInput")
    e2g_d = nc.dram_tensor("e2g", [D, G3], F32, kind="ExternalInput")
    linw_d = nc.dram_tensor("linw", [HA, D], F32, kind="ExternalInput")
    gi0_d = nc.dram_tensor("gi0", [128, 9 * BL], F32, kind="ExternalInput")
    id8_d = nc.dram_tensor("id8", [BL, BL], F32, kind="ExternalInput")
    ones_d = nc.dram_tensor("ones1", [1, BL], F32, kind="ExternalInput")
    onesb_d = nc.dram_tensor("ones1b", [1, BL], enc_dt, kind="ExternalInput")
    out_d = nc.dram_tensor("logits", [BL, l_steps * D], F32,
                           kind="ExternalOutput")

    with ExitStack() as ctx:
        tc = ctx.enter_context(tile.TileContext(nc))
        cpool = ctx.enter_context(tc.tile_pool(name="const", bufs=1))
        gpool = ctx.enter_context(tc.tile_pool(name="gi", bufs=1))
        tpool = ctx.enter_context(tc.tile_pool(name="tmp", bufs=3))
        ps1 = ctx.enter_context(tc.tile_pool(name="ps1", bufs=1, space="PSUM"))
        ps2 = ctx.enter_context(tc.tile_pool(name="ps2", bufs=2, space="PSUM"))

        def ctile(shape, dt, name):
            return cpool.tile(shape, dt, tag=name, name=name)

        # ---- persistent SBUF tensors ----
        xtc = [[ctile([128, chbl], enc_dt, f"xtc{p}_{j}") for j in range(3)]
               for p in range(2)]
        wih_t = [ctile([128, G3], enc_dt, f"wih{j}") for j in range(3)]
        whh_t = [ctile([128, G3], enc_dt, f"whh{j}") for j in range(3)]
        whhd_t = [ctile([128, G3], F32, f"whhd{j}") for j in range(3)]
        e2g_t = ctile([128, G3], F32, "e2g")
        linw_t = [ctile([128, D], F32, f"linw{j}") for j in range(3)]
        gi0_t = ctile([128, 9 * BL], F32, "gi0")
        id8_t = ctile([BL, BL], F32, "id8")
        lout = ctile([BL, l_steps * D], F32, "lout")
        hh = [ctile([128, 3 * BL], enc_dt, f"h{p}") for p in range(2)]
        hhd = [ctile([128, 3 * BL], F32, f"hd{p}") for p in range(2)]
        gis = [gpool.tile([128, 9 * chbl], enc_dt, tag=f"gi{p}", name=f"gi{p}")
               for p in range(2)]
        ohT_s = ctile([128, BL], F32, "ohT")

        # ---- weight DMAs ----
        for j, (ks, pc) in enumerate(KC):
            nc.sync.dma_start(wih_t[j][:pc, :], wih_d[ks:ks + pc, :])
            nc.sync.dma_start(whh_t[j][:pc, :], whh_d[ks:ks + pc, :])
            nc.sync.dma_start(whhd_t[j][:pc, :], whhd_d[ks:ks + pc, :])
            nc.sync.dma_start(linw_t[j][:pc, :], linw_d[ks:ks + pc, :])
        nc.sync.dma_start(e2g_t[:, :], e2g_d[:, :])
        nc.sync.dma_start(gi0_t[:, :], gi0_d[:, :])
        nc.sync.dma_start(id8_t[:, :], id8_d[:, :])

        def dma_x_chunk(c):
            for j, (ks, pc) in enumerate(KC):
                nc.sync.dma_start(
                    xtc[c % 2][j][:pc, :],
                    xt_d[ks:ks + pc, c * chbl:(c + 1) * chbl])

        # ---- initial state: h = 0, ones in the augmented row ----
        for p in range(2):
            nc.vector.memset(hh[p][:, :], 0.0)
            nc.sync.dma_start(hh[p][48:49, 16:24], onesb_d[:, :])
            nc.vector.memset(hhd[p][:, :], 0.0)
            nc.sync.dma_start(hhd[p][48:49, 16:24], ones_d[:, :])

        # ---- gi production: one (nt, band) group = 3 MMs + 1 copy ----
        pw = min(512, chbl)
        n_nt = chbl // pw

        def prod_group(src_chunk, nt, bi):
            g, mj = bi // 3, bi % 3
            ms, mcn = MC[mj]
            dst = gis[(src_chunk) % 2]
            ps = ps2.tile([128, pw], F32, tag="prod", name="prod")
            for j, (ks, pc) in enumerate(KC):
                nc.tensor.matmul(
                    ps[:mcn, :],
                    wih_t[j][:pc, g * H + ms:g * H + ms + mcn],
                    xtc[src_chunk % 2][j][:pc, nt * pw:nt * pw + pw],
                    start=(j == 0), stop=(j == 2))
            nc.vector.tensor_copy(
                dst[:mcn, bi * chbl + nt * pw:bi * chbl + nt * pw + pw],
                ps[:mcn, :])

        prod_groups = [(nt, bi) for nt in range(n_nt) for bi in range(9)]

        # prologue: x chunk 0 + produce chunk 0
        dma_x_chunk(0)
        if n_chunks > 1:
            dma_x_chunk(1)
        for nt, bi in prod_groups:
            prod_group(0, nt, bi)

        c3 = lambda ap: ap.rearrange("p (a c) -> p a c", a=3)

        # ---- one GRU step (encoder flavor: gi from SBUF buffer) ----
        # Split into part A (M-chunks 0,1) and part B (M-chunk 2): part A's
        # gate math overlaps part B's matmuls, so the serial tail is only the
        # short part-B chain and the next step's matmuls start ~immediately.
        def enc_step(c, sl):
            s = c * ch + sl
            par = s % 2
            h_in, h_out = hh[par], hh[1 - par]
            gi = gis[c % 2]
            ps_r = ps1.tile([128, 3 * BL], F32, tag="ps_r", name="ps_r")
            ps_z = ps1.tile([128, 3 * BL], F32, tag="ps_z", name="ps_z")
            ps_n = ps1.tile([128, 3 * BL], F32, tag="ps_n", name="ps_n")
            gi9 = gi[:, :].rearrange("p (bi c) -> p bi c", bi=9)

            def mm_part(mjs):
                for g, pst in ((0, ps_r), (2, ps_n), (1, ps_z)):
                    for mj in mjs:
                        ms, mcn = MC[mj]
                        for j, (ks, pc) in enumerate(KC):
                            nc.tensor.matmul(
                                pst[:mcn, mj * BL:(mj + 1) * BL],
                                whh_t[j][:pc, g * H + ms:g * H + ms + mcn],
                                h_in[:pc, j * BL:(j + 1) * BL],
                                start=(j == 0), stop=(j == 2))

            def gates_part(tag, cols, pn, bsl, hp):
                cw = cols.stop - cols.start
                c2 = lambda ap: ap.rearrange("p (a c) -> p a c", a=pn)
                tt = {}
                for nm_ in ("t_r", "t_z", "t_n", "r_s", "z_s", "n_s",
                            "d_t", "e_t"):
                    tt[nm_] = tpool.tile([128, cw], F32, tag=f"{nm_}{tag}",
                                         name=f"{nm_}{tag}")
                gsl = lambda g0: gi9[:, g0 * 3 + bsl.start:g0 * 3 + bsl.stop,
                                     sl * BL:(sl + 1) * BL]
                nc.vector.tensor_add(c2(tt["t_r"][:, :]),
                                     c2(ps_r[:, cols]), gsl(0))
                nc.scalar.activation(tt["r_s"][:, :], tt["t_r"][:, :], SIG)
                nc.vector.tensor_mul(tt["t_n"][:, :], tt["r_s"][:, :],
                                     ps_n[:, cols])
                nc.vector.tensor_add(c2(tt["t_n"][:, :]),
                                     c2(tt["t_n"][:, :]), gsl(2))
                nc.scalar.activation(tt["n_s"][:, :], tt["t_n"][:, :], TANH)
                nc.vector.tensor_sub(tt["d_t"][:, :], h_in[:, cols],
                                     tt["n_s"][:, :])
                nc.vector.tensor_add(c2(tt["t_z"][:, :]),
                                     c2(ps_z[:, cols]), gsl(1))
                nc.scalar.activation(tt["z_s"][:, :], tt["t_z"][:, :], SIG)
                nc.vector.tensor_mul(tt["e_t"][:, :], tt["z_s"][:, :],
                                     tt["d_t"][:, :])
                nc.vector.tensor_add(h_out[:hp, cols],
                                     tt["n_s"][:hp, :], tt["e_t"][:hp, :])

            mm_part((0, 1))
            gates_part("A", slice(0, 2 * BL), 2, slice(0, 2), 128)
            mm_part((2,))
            gates_part("B", slice(2 * BL, 3 * BL), 1, slice(2, 3), MC[2][1])

        # ---- encoder ----
        for c in range(n_chunks):
            if c + 2 < n_chunks:
                dma_x_chunk(c + 2)
            for sl in range(ch):
                enc_step(c, sl)
                if c + 1 < n_chunks:
                    for nt, bi in prod_groups[sl::ch]:
                        prod_group(c + 1, nt, bi)

        # ---- decoder (fp32): hand off encoder h into f32 tiles ----
        nc.vector.tensor_copy(hhd[0][:, :], hh[t_steps % 2][:, :])
        for l in range(l_steps):
            par = l % 2
            h_in, h_out = hhd[par], hhd[1 - par]
            ps_r = ps1.tile([128, 3 * BL], F32, tag="ps_r", name="ps_r")
            ps_z = ps1.tile([128, 3 * BL], F32, tag="ps_z", name="ps_z")
            ps_n = ps1.tile([128, 3 * BL], F32, tag="ps_n", name="ps_n")
            ps_ne = ps1.tile([128, 3 * BL], F32, tag="ps_ne", name="ps_ne")
            # n-gate gh first: 9 matmuls with no one-hot dependency cover the
            # previous step's argmax->one-hot chain latency
            for g, pst in ((2, ps_n), (0, ps_r), (1, ps_z)):
                for mj, (ms, mcn) in enumerate(MC):
                    for j, (ks, pc) in enumerate(KC):
                        nc.tensor.matmul(
                            pst[:mcn, mj * BL:(mj + 1) * BL],
                            whhd_t[j][:pc, g * H + ms:g * H + ms + mcn],
                            h_in[:pc, j * BL:(j + 1) * BL],
                            start=(j == 0),
                            stop=(j == 2 and (l == 0 or g == 2)))
                    if l > 0 and g < 2:
                        nc.tensor.matmul(
                            pst[:mcn, mj * BL:(mj + 1) * BL],
                            e2g_t[:, g * H + ms:g * H + ms + mcn],
                            ohT_s[:, :],
                            start=False, stop=True)
                if l > 0 and g == 0:
                    for mj, (ms, mcn) in enumerate(MC):
                        nc.tensor.matmul(
                            ps_ne[:mcn, mj * BL:(mj + 1) * BL],
                            e2g_t[:, 2 * H + ms:2 * H + ms + mcn],
                            ohT_s[:, :],
                            start=True, stop=True)

            t_r = tpool.tile([128, 3 * BL], F32, tag="t_r", name="t_r")
            t_z = tpool.tile([128, 3 * BL], F32, tag="t_z", name="t_z")
            t_n = tpool.tile([128, 3 * BL], F32, tag="t_n", name="t_n")
            r_s = tpool.tile([128, 3 * BL], F32, tag="r_s", name="r_s")
            z_s = tpool.tile([128, 3 * BL], F32, tag="z_s", name="z_s")
            n_s = tpool.tile([128, 3 * BL], F32, tag="n_s", name="n_s")
            d_t = tpool.tile([128, 3 * BL], F32, tag="d_t", name="d_t")
            e_t = tpool.tile([128, 3 * BL], F32, tag="e_t", name="e_t")
            if l == 0:
                nc.vector.tensor_add(t_r[:, :], ps_r[:, :], gi0_t[:, 0:24])
                nc.scalar.activation(r_s[:, :], t_r[:, :], SIG)
                nc.vector.tensor_mul(t_n[:, :], r_s[:, :], ps_n[:, :])
                nc.vector.tensor_add(t_n[:, :], t_n[:, :], gi0_t[:, 48:72])
                nc.scalar.activation(n_s[:, :], t_n[:, :], TANH)
                nc.vector.tensor_sub(d_t[:, :], h_in[:, :], n_s[:, :])
                nc.vector.tensor_add(t_z[:, :], ps_z[:, :], gi0_t[:, 24:48])
                nc.scalar.activation(z_s[:, :], t_z[:, :], SIG)
            else:
                nc.scalar.activation(r_s[:, :], ps_r[:, :], SIG)
                nc.vector.tensor_mul(t_n[:, :], r_s[:, :], ps_n[:, :])
                nc.vector.tensor_add(t_n[:, :], t_n[:, :], ps_ne[:, :])
                nc.scalar.activation(n_s[:, :], t_n[:, :], TANH)
                nc.vector.tensor_sub(d_t[:, :], h_in[:, :], n_s[:, :])
                nc.scalar.activation(z_s[:, :], ps_z[:, :], SIG)
            nc.vector.tensor_mul(e_t[:, :], z_s[:, :], d_t[:, :])
            nc.vector.tensor_add(h_out[:, 0:2 * BL], n_s[:, 0:2 * BL],
                                 e_t[:, 0:2 * BL])
            nc.vector.tensor_add(h_out[:MC[2][1], 2 * BL:3 * BL],
                                 n_s[:MC[2][1], 2 * BL:3 * BL],
                                 e_t[:MC[2][1], 2 * BL:3 * BL])

            # logits = h_out_aug @ linW_aug.T  -> [BL, D]
            ps_log = ps1.tile([BL, D], F32, tag="ps_log", name="ps_log")
            for j, (ks, pc) in enumerate(KC):
                nc.tensor.matmul(ps_log[:, :], h_out[:pc, j * BL:(j + 1) * BL],
                                 linw_t[j][:pc, :], start=(j == 0),
                                 stop=(j == 2))
            nc.vector.tensor_copy(lout[:, l * D:(l + 1) * D], ps_log[:, :])
            if l + 1 < l_steps:
                max8 = tpool.tile([BL, 8], F32, tag="max8", name="max8")
                oh_s = tpool.tile([BL, D], F32, tag="oh", name="oh")
                nc.vector.max(max8[:, :], lout[:, l * D:(l + 1) * D])
                nc.vector.tensor_scalar(
                    oh_s[:, :], lout[:, l * D:(l + 1) * D], max8[:, 0:1],
                    None, op0=mybir.AluOpType.is_equal)
                ps_oht = ps1.tile([128, BL], F32, tag="ps_oht", name="ps_oht")
                nc.tensor.transpose(ps_oht[:, :], oh_s[:, :], id8_t[:, :])
                nc.vector.tensor_copy(ohT_s[:, :], ps_oht[:, :])

        nc.sync.dma_start(out_d[:, :], lout[:, :])

    nc.compile()
    return nc


def prep_inputs(x, target, emb, enc_Wih, enc_Whh, enc_bih, enc_bhh,
                dec_Wih, dec_Whh, dec_bih, dec_bhh, lin_W, lin_b,
                t_steps=T, l_steps=L):
    """Build per-core input maps (host-side sharding + layout)."""
    x = np.asarray(x, np.float32)
    target = np.asarray(target)
    emb = np.asarray(emb, np.float32)
    enc_Wih = np.asarray(enc_Wih, np.float32)
    enc_Whh = np.asarray(enc_Whh, np.float32)
    enc_bih = np.asarray(enc_bih, np.float32)
    enc_bhh = np.asarray(enc_bhh, np.float32)
    dec_Wih = np.asarray(dec_Wih, np.float32)
    dec_Whh = np.asarray(dec_Whh, np.float32)
    dec_bih = np.asarray(dec_bih, np.float32)
    dec_bhh = np.asarray(dec_bhh, np.float32)
    lin_W = np.asarray(lin_W, np.float32)
    lin_b = np.asarray(lin_b, np.float32)

    bias_rzn = enc_bih.copy()
    bias_rzn[:2 * H] += enc_bhh[:2 * H]
    wihT = np.concatenate([enc_Wih.T, bias_rzn[None, :]], 0)
    whh_aug = np.zeros((1, G3), np.float32)
    whh_aug[0, 2 * H:] = enc_bhh[2 * H:]
    whhT = np.concatenate([enc_Whh.T, whh_aug], 0)
    whhd_aug = np.zeros((1, G3), np.float32)
    whhd_aug[0, 2 * H:] = dec_bhh[2 * H:]
    whhdT = np.concatenate([dec_Whh.T, whhd_aug], 0)
    dbias = dec_bih.copy()
    dbias[:2 * H] += dec_bhh[:2 * H]
    e2g_f32 = (emb.astype(np.float64) @ dec_Wih.T.astype(np.float64)
               + dbias.astype(np.float64)).astype(np.float32)
    linT = np.concatenate([lin_W.T, lin_b[None, :]], 0)
    id8 = np.eye(BL, dtype=np.float32)

    in_maps = []
    for cix in range(NCORES):
        bs = slice(cix * BL, (cix + 1) * BL)
        xa = x[bs, :t_steps, :]  # [BL, T, F]
        xa_t = np.ascontiguousarray(xa.transpose(2, 1, 0)).reshape(F, -1)
        xa_t = np.concatenate(
            [xa_t, np.ones((1, t_steps * BL), np.float32)], 0)
        toks = target[bs, 0, 0].astype(np.int64)
        gi0_full = e2g_f32[toks]  # [BL, G3]
        gi0 = np.zeros((128, 9 * BL), np.float32)
        for g in range(3):
            for mj, (ms, mcn) in enumerate(MC):
                gi0[:mcn, (g * 3 + mj) * BL:(g * 3 + mj + 1) * BL] = \
                    gi0_full[:, g * H + ms:g * H + ms + mcn].T
        in_maps.append({
            "xt": np.ascontiguousarray(xa_t).astype(NPBF16),
            "wih": wihT.astype(NPBF16), "whh": whhT.astype(NPBF16),
            "whhd": whhdT,
            "e2g": e2g_f32, "linw": linT,
            "gi0": gi0, "id8": id8,
            "ones1": np.ones((1, BL), np.float32),
            "ones1b": np.ones((1, BL), NPBF16),
        })
    return in_maps


_NC_CACHE = {}


def run_model(x, target, emb, enc_Wih, enc_Whh, enc_bih, enc_bhh,
              dec_Wih, dec_Whh, dec_bih, dec_bhh, lin_W, lin_b,
              t_steps=T, l_steps=L, trace=False):
    key = (t_steps, l_steps)
    if key not in _NC_CACHE:
        _NC_CACHE[key] = build_kernel(t_steps, l_steps)
    nc = _NC_CACHE[key]
    in_maps = prep_inputs(x, target, emb, enc_Wih, enc_Whh, enc_bih, enc_bhh,
                          dec_Wih, dec_Whh, dec_bih, dec_bhh, lin_W, lin_b,
                          t_steps=t_steps, l_steps=l_steps)
    res = run_bass_kernel_spmd(nc, in_maps, core_ids=list(range(NCORES)),
                               trace=trace)
    logits = np.stack([res.results[i]["logits"] for i in range(NCORES)])
    logits = logits.reshape(NCORES * BL, l_steps, D).astype(np.float32)
    return logits, res


def kernel(x, target, emb, enc_Wih, enc_Whh, enc_bih, enc_bhh,
           dec_Wih, dec_Whh, dec_bih, dec_bhh, lin_W, lin_b):
    target = np.asarray(target)
    logits, _ = run_model(x, target, emb, enc_Wih, enc_Whh, enc_bih, enc_bhh,
                          dec_Wih, dec_Whh, dec_bih, dec_bhh, lin_W, lin_b)
    softmaxs = logits  # [B, L, D]
    softmax_cal = softmaxs[:, :-1, :].reshape(-1, D)
    target_cal = target[:, 1:, :].reshape(-1)
    asr_outputs = np.argmax(softmaxs[:, :-1, :], axis=2)[:, :, None]
    asr_outputs = asr_outputs.astype(np.int32)
    return softmax_cal, target_cal, asr_outputs
